# revision 2
# baseline (speedup 1.0000x reference)
"""Trainium2 Bass kernel for nn_Classifier0 (quadrant-sum classifier).

reference:
    agg[n, q]  = quadrant sums of x[n] (512x512, quadrants of 256x256)
    w          = g * v[..., 0] / ||v||            [4, 4]
    y          = agg[:, :, None] * w + b_fgl      [N, 4, 4]
    out        = y.reshape(N, 16) @ W_fc.T + b_fc [N, 10]

Algebraic refactor (exact in real arithmetic):
    out[n, c] = sum_q agg[n, q] * A[q, c] + cc[c]
      A[q, c] = sum_j w[q, j] * W_fc[c, 4q + j]         (4 x 10, host, fp64)
      cc[c]   = b_fgl.ravel() @ W_fc[c] + b_fc[c]       (10, host, fp64)

Device work (data-parallel, 32 samples per core, 8 chunks of 4 samples):
  - 7 chunks (samples 0..27): one contiguous 4 MB DMA each into a
    [128, 8192] tile (partition p: sample p//32, image rows (p%32)*16..+16,
    top half iff p%32 < 16).  Full 128-partition transfers keep the HWDGE
    descriptor->engine dealing aligned with the fixed SBUF port map
    (16 x 16 KB descriptors per engine at ~27 GB/s each, 99% of the 27.2
    GB/s port rate).  Partial-partition transfers are dealt positionally
    from engine 0 and break that alignment — measured ~35% global
    throughput loss from crossbar contention — so everything here is
    full-width.  Chunk 0 is split across both HWDGE rings (sync + scalar)
    so the 16 SDMA engines spin up sooner.
  - DVE tensor_reduce sums the left 256 columns per partition-row, ACT
    (in-place Copy with accum_out) the right 256.
  - chunk 7 uses the same C=4 masks but is split into tapering pieces
    [128, 8 rows] + [4 rows] + [3 rows] + two final [128, 256] half-row
    pieces (right then left, both on DVE), each with its own [128, 1] sum
    column, so the post-stream serial chain (reduce -> matmul -> copy ->
    y DMA) is ~1.5-2 us.
  - Contraction is two PSUM accumulation groups with shared [128, 40]
    zero-masked weights: psumM [7, 40] for chunks 0-6 (bias + 2 matmuls,
    copied + DMA'd hidden under the stream) and psumT [1, 40] for the
    chunk-7 pieces (bias + 8 single-column matmuls, mostly hidden; the
    final y row is a single 160 B descriptor).

Per-core stream is SBUF-AXI-port bound: 16 SDMA engines x ~27 GB/s ->
33.6 MB in ~79.4 us.  Measured exec adds ~3 us pre-DMA latency, ~1.5-2 us
tail and ~8.5 us of fixed NEFF epilogue (a full-semaphore-file sweep).
On some executions SDMA engine 15 degrades 10-30% (~+10-20 us); transfer
shapes cannot counter it without breaking port alignment (measured), so
this kernel optimizes the clean path.
"""

import numpy as np

N, S = 256, 512
H = S // 2
NCORES = 8
SPC = N // NCORES  # samples per core (32)
NCLS = 10

C = 4  # samples per chunk
NCH = 8  # chunks per core
RPP = 16  # rows per partition per chunk
P7A, P7B, P7C = 8, 4, 3  # chunk-7 piece rows (then 1 final row, split L/R)

_PROGRAM_CACHE = {}


def _build_program():
    from contextlib import ExitStack

    import concourse.bacc as bacc
    import concourse.mybir as mybir
    import concourse.tile as tile

    nc = bacc.Bacc("TRN2", target_bir_lowering=False, debug=False)
    dt = mybir.dt.float32

    xl_t = nc.dram_tensor("xl", [NCH - 1, 128, RPP * S], dt, kind="ExternalInput")
    x7_t = nc.dram_tensor("x7", [128, (RPP - 1) * S], dt, kind="ExternalInput")
    xf_t = nc.dram_tensor("xf", [2, 128, H], dt, kind="ExternalInput")
    # cols 0:40 walm, 40:80 warm; row 0 cols 80:120 = cc (bias, tiled x4)
    cst_t = nc.dram_tensor("cst", [128, 120], dt, kind="ExternalInput")
    y_t = nc.dram_tensor("y", [SPC, NCLS], dt, kind="ExternalOutput")

    with tile.TileContext(nc) as tc, ExitStack() as ctx:
        xpool = ctx.enter_context(tc.tile_pool(name="xp", bufs=8))
        cpool = ctx.enter_context(tc.tile_pool(name="cp", bufs=1))
        ppool = ctx.enter_context(tc.tile_pool(name="pp", bufs=1, space="PSUM"))

        xl_ap = xl_t.ap()
        xf_ap = xf_t.ap()
        y2 = y_t.ap().rearrange("(k j) c -> k (j c)", j=C)  # [8, 40]

        bufL = cpool.tile([128, NCH - 1], dt)
        bufR = cpool.tile([128, NCH - 1], dt)
        # chunk-7 piece sums: single columns, no masking needed
        l7aL = cpool.tile([128, 1], dt)
        l7aR = cpool.tile([128, 1], dt)
        l7bL = cpool.tile([128, 1], dt)
        l7bR = cpool.tile([128, 1], dt)
        l7cL = cpool.tile([128, 1], dt)
        l7cR = cpool.tile([128, 1], dt)
        finL = cpool.tile([128, 1], dt)
        finR = cpool.tile([128, 1], dt)
        ones7 = cpool.tile([1, NCH - 1], dt)
        nc.vector.memset(ones7[:], 1.0)
        # one constant load on the scalar ring, after the chunk-0 half
        cst = cpool.tile([128, 120], dt)
        walm, warm = cst[:, 0:40], cst[:, 40:80]
        ccbt = cst[0:1, 80:120]

        for k in range(NCH - 1):
            xt = xpool.tile([128, RPP * S], dt, tag="xl", bufs=4)
            if k == 0:
                hf = RPP * S // 2
                nc.sync.dma_start(xt[:, 0:hf], xl_ap[0][:, 0:hf])
                nc.scalar.dma_start(xt[:, hf:], xl_ap[0][:, hf:])
                nc.scalar.dma_start(cst[:], cst_t.ap())
            else:
                nc.sync.dma_start(xt[:], xl_ap[k])
            xv = xt[:].rearrange("p (r c) -> p r c", c=S)
            nc.vector.tensor_reduce(
                bufL[:, k : k + 1],
                xv[:, :, 0:H],
                axis=mybir.AxisListType.XY,
                op=mybir.AluOpType.add,
            )
            nc.scalar.activation(
                xv[:, :, H:S],
                xv[:, :, H:S],
                mybir.ActivationFunctionType.Copy,
                accum_out=bufR[:, k : k + 1],
            )

        # chunk 7: one [128, 15 rows] tile filled by three tapering
        # full-width DMAs; each piece reduced as it lands
        WA, WB = P7A * S, (P7A + P7B) * S
        xt7 = xpool.tile([128, (RPP - 1) * S], dt, tag="x7", bufs=1)
        nc.sync.dma_start(xt7[:, 0:WA], x7_t.ap()[:, 0:WA])
        nc.sync.dma_start(xt7[:, WA:WB], x7_t.ap()[:, WA:WB])
        nc.sync.dma_start(xt7[:, WB:], x7_t.ap()[:, WB:])
        xv7 = xt7[:].rearrange("p (r c) -> p r c", c=S)
        for (r0, r1), (bL, bR) in [
            ((0, P7A), (l7aL, l7aR)),
            ((P7A, P7A + P7B), (l7bL, l7bR)),
            ((P7A + P7B, RPP - 1), (l7cL, l7cR)),
        ]:
            nc.vector.tensor_reduce(
                bL[:, 0:1],
                xv7[:, r0:r1, 0:H],
                axis=mybir.AxisListType.XY,
                op=mybir.AluOpType.add,
            )
            nc.scalar.activation(
                xv7[:, r0:r1, H:S],
                xv7[:, r0:r1, H:S],
                mybir.ActivationFunctionType.Copy,
                accum_out=bR[:, 0:1],
            )

        # final half-row pieces: right then left, both reduced on DVE so the
        # ACT accumulator-read latency stays off the critical tail
        ftR = xpool.tile([128, H], dt, tag="fR", bufs=1)
        nc.sync.dma_start(ftR[:], xf_ap[0])
        nc.vector.tensor_reduce(
            finR[:, 0:1],
            ftR[:].rearrange("p (r c) -> p r c", c=H),
            axis=mybir.AxisListType.XY,
            op=mybir.AluOpType.add,
        )
        ftL = xpool.tile([128, H], dt, tag="fL", bufs=1)
        nc.sync.dma_start(ftL[:], xf_ap[1])
        nc.vector.tensor_reduce(
            finL[:, 0:1],
            ftL[:].rearrange("p (r c) -> p r c", c=H),
            axis=mybir.AxisListType.XY,
            op=mybir.AluOpType.add,
        )

        # chunks 0-6: bias + two masked matmuls, copy + y DMA — all ready
        # before the stream drains (hidden)
        psumM = ppool.tile([NCH - 1, C * NCLS], dt)
        nc.tensor.matmul(psumM[:], lhsT=ones7[:], rhs=ccbt, start=True, stop=False)
        nc.tensor.matmul(psumM[:], lhsT=bufL[:], rhs=walm, start=False, stop=False)
        nc.tensor.matmul(psumM[:], lhsT=bufR[:], rhs=warm, start=False, stop=True)
        outM = cpool.tile([NCH - 1, C * NCLS], dt)
        nc.vector.tensor_copy(outM[:], psumM[:])
        nc.sync.dma_start(y2[0 : NCH - 1, :], outM[:])

        # chunk 7: bias + 8 single-column matmuls into a [1, 40] group; only
        # the last pieces' matmuls + copy + one 160 B y-row DMA are serial
        ones1 = ones7[:, 0:1]
        psumT = ppool.tile([1, C * NCLS], dt)
        nc.tensor.matmul(psumT[:], lhsT=ones1, rhs=ccbt, start=True, stop=False)
        for bL, bR in [(l7aL, l7aR), (l7bL, l7bR), (l7cL, l7cR)]:
            nc.tensor.matmul(psumT[:], lhsT=bL[:], rhs=walm, start=False, stop=False)
            nc.tensor.matmul(psumT[:], lhsT=bR[:], rhs=warm, start=False, stop=False)
        nc.tensor.matmul(psumT[:], lhsT=finR[:], rhs=warm, start=False, stop=False)
        nc.tensor.matmul(psumT[:], lhsT=finL[:], rhs=walm, start=False, stop=True)
        outT = cpool.tile([1, C * NCLS], dt)
        nc.vector.tensor_copy(outT[:], psumT[:])
        nc.sync.dma_start(y2[NCH - 1 : NCH, :], outT[:])

    nc.compile()
    return nc


def _host_params(v, g, b_fgl, W_fc, b_fc):
    """Fold the tiny params into zero-masked walm/warm [128, 40], cc [1, 40]."""
    v64 = v.astype(np.float64)
    w = g.astype(np.float64) * (v64[..., 0] / np.linalg.norm(v64, axis=-1))  # [4,4]
    A = np.einsum("qj,cqj->qc", w, W_fc.astype(np.float64).reshape(NCLS, 4, 4))
    cc = b_fgl.astype(np.float64).reshape(-1) @ W_fc.astype(np.float64).T
    cc = cc + b_fc.astype(np.float64)

    # quadrant ids: 0=TL, 1=BL, 2=BR, 3=TR; partition p: sample slot p//32,
    # top half iff p%32 < 16 (16 consecutive image rows per partition)
    p = np.arange(128)
    top = (p % 32) < 16
    al = np.where(top[:, None], A[0][None, :], A[1][None, :])  # [128,10]
    ar = np.where(top[:, None], A[3][None, :], A[2][None, :])
    grp = p // 32
    walm = np.zeros((128, C * NCLS))
    warm = np.zeros((128, C * NCLS))
    for j in range(C):
        sel = grp == j
        walm[sel, j * NCLS : (j + 1) * NCLS] = al[sel]
        warm[sel, j * NCLS : (j + 1) * NCLS] = ar[sel]
    ccb = np.tile(cc, C).reshape(1, C * NCLS)
    return (
        np.ascontiguousarray(walm, dtype=np.float32),
        np.ascontiguousarray(warm, dtype=np.float32),
        np.ascontiguousarray(ccb, dtype=np.float32),
    )


def _pack_core(xc):
    """xc [32, 512, 512] -> packed DMA source tensors."""
    # [chunk, partition, row-in-partition, col] — pure reshape of xc
    full = xc.reshape(NCH, C, 32, RPP, S).reshape(NCH, 128, RPP, S)
    xl = full[: NCH - 1].reshape(NCH - 1, 128, RPP * S)  # contiguous view
    x7 = np.ascontiguousarray(full[NCH - 1, :, 0 : RPP - 1, :]).reshape(
        128, (RPP - 1) * S
    )
    fin = full[NCH - 1, :, RPP - 1, :]  # [128, 512]
    xf = np.ascontiguousarray(np.stack([fin[:, H:S], fin[:, 0:H]], 0))  # right, left
    return {"xl": xl, "x7": x7, "xf": xf}


def _run(inputs, trace=False):
    from concourse.bass_utils import run_bass_kernel_spmd

    if "nc" not in _PROGRAM_CACHE:
        _PROGRAM_CACHE["nc"] = _build_program()
    nc = _PROGRAM_CACHE["nc"]

    x = np.ascontiguousarray(np.asarray(inputs["x"], dtype=np.float32))
    walm, warm, ccb = _host_params(
        np.asarray(inputs["v"], np.float32),
        np.asarray(inputs["g"], np.float32),
        np.asarray(inputs["b_fgl"], np.float32),
        np.asarray(inputs["W_fc"], np.float32),
        np.asarray(inputs["b_fc"], np.float32),
    )
    cst = np.zeros((128, 120), np.float32)
    cst[:, 0:40] = walm
    cst[:, 40:80] = warm
    cst[0, 80:120] = ccb[0]

    x_sh = x.reshape(NCORES, SPC, S, S)
    in_maps = []
    for i in range(NCORES):
        m = _pack_core(x_sh[i])
        m["cst"] = cst
        in_maps.append(m)
    res = run_bass_kernel_spmd(nc, in_maps, list(range(NCORES)), trace=trace)
    y = np.concatenate([res.results[i]["y"] for i in range(NCORES)], axis=0)
    return y, res.exec_time_ns


def kernel(**inputs) -> np.ndarray:
    y, _ = _run(inputs, trace=False)
    return y
